# revision 1
# baseline (speedup 1.0000x reference)
"""GraphSAGE (3x SAGEConv mean-agg + linear classifier + log_softmax) on 8
Trainium2 NeuronCores via Bass.

Self-contained: host-side packing + SPMD bass program + gather/unshard.

Sharding: nodes are dst-sharded 8 ways (core c owns nodes [c*NP, (c+1)*NP)).
Per layer, each core:
  - gathers its in-edges' source rows from a replicated DRAM feature table
    with [128,1]-offset indirect DMAs (one 128-token grid column per call),
  - segment-reduces degree-sorted "prow" rectangles on the Vector engine
    (one strided tensor_reduce per rectangle), scales by 1/deg,
  - PE-transposes the mean grid to feature-major, matmuls Wl/Wr with PSUM
    accumulation, applies bias+ReLU on the Scalar engine,
  - PE-transposes back to node-major and AllGathers the shard into the next
    layer's table (compute/slot order).
The tiny 64-wide linears are replicated on every core.
"""
import os
import sys
import types

import numpy as np

sys.path.insert(0, "/opt/trn_rl_repo")

P = 8
F = 64
NCLS = 10
CW = 64          # gather-chunk width (grid columns per SBUF chunk buffer)
MMW = 512        # matmul moving-chunk width

LAST_EXEC_NS = None


# ---------------------------------------------------------------- host packing

def _build_meta(edge_index, n_nodes):
    N = n_nodes
    NP = N // P
    PR = (NP + 127) // 128
    SLOTS = PR * 128
    assert NP < SLOTS, "need at least one dummy slot (NP not divisible by 128)"
    src = np.asarray(edge_index[0], dtype=np.int64)
    dst = np.asarray(edge_index[1], dtype=np.int64)
    deg = np.bincount(dst, minlength=N).astype(np.int64)

    orders = []
    deg_sorted = []
    for c in range(P):
        dl = deg[c * NP:(c + 1) * NP]
        o = np.argsort(-dl, kind="stable")
        orders.append(o)
        deg_sorted.append(dl[o])
    w_prow = np.zeros(PR, dtype=np.int64)
    for i in range(PR):
        w_prow[i] = max(1, max(int(ds[i * 128]) for ds in deg_sorted))

    chunks = []
    col = 0
    cur = {"c0": 0, "width": 0, "rects": []}
    i = 0
    while i < PR:
        w = int(w_prow[i])
        if cur["width"] + w > CW and cur["width"] > 0:
            chunks.append(cur)
            cur = {"c0": col, "width": 0, "rects": []}
        m = 0
        while (i + m < PR and int(w_prow[i + m]) == w
               and cur["width"] + (m + 1) * w <= CW):
            m += 1
        if m == 0:
            assert w <= CW, f"prow width {w} exceeds chunk width {CW}"
            m = 1
        cur["rects"].append({"coff": cur["width"], "i0": i, "m": m, "w": w})
        cur["width"] += m * w
        col += m * w
        i += m
    if cur["width"] > 0:
        chunks.append(cur)
    C = col

    cc = 0
    for ch in chunks:
        ch["c0"] = cc
        cc += ch["width"]
    c0_prow = np.zeros(PR, dtype=np.int64)
    for ch in chunks:
        for r in ch["rects"]:
            for k in range(r["m"]):
                c0_prow[r["i0"] + k] = ch["c0"] + r["coff"] + k * r["w"]

    grids = []
    slot_nodes = []
    slot_of = np.full(N, -1, dtype=np.int64)
    for c in range(P):
        o = orders[c]
        slot_node = np.full(SLOTS, -1, dtype=np.int64)
        slot_node[:NP] = o + c * NP
        slot_nodes.append(slot_node)
        slot_of[o + c * NP] = np.arange(NP)

        grid = np.full((128, C), -1, dtype=np.int64)
        m = (dst >= c * NP) & (dst < (c + 1) * NP)
        es, ed = src[m], dst[m] - c * NP
        eo = np.argsort(ed, kind="stable")
        es, ed = es[eo], ed[eo]
        estart = np.zeros(NP + 1, dtype=np.int64)
        np.cumsum(np.bincount(ed, minlength=NP), out=estart[1:])
        r_e = slot_of[ed + c * NP]
        k_e = np.arange(es.shape[0]) - estart[ed]
        pp = r_e % 128
        cols = c0_prow[r_e // 128] + k_e
        grid[pp, cols] = es
        grids.append(grid)

    return {
        "N": N, "NP": NP, "PR": PR, "SLOTS": SLOTS, "C": C,
        "chunks": chunks, "grids": grids, "slot_nodes": slot_nodes,
        "slot_of": slot_of, "deg": deg,
    }


def _build_core_inputs(meta, x):
    N, NP, PR, SLOTS, C = (meta[k] for k in ("N", "NP", "PR", "SLOTS", "C"))
    Z1 = N
    Z2 = NP          # core 0's first dummy slot (zeroed on device)
    inv = 1.0 / np.maximum(meta["deg"], 1).astype(np.float32)

    htbl = np.zeros((N + 128, F), np.float32)
    htbl[:N] = x

    per_core = []
    for c in range(P):
        grid = meta["grids"][c]
        off1 = np.where(grid >= 0, grid, Z1).astype(np.int32)
        g2 = np.where(grid >= 0,
                      (grid // NP) * SLOTS + meta["slot_of"][np.maximum(grid, 0)],
                      Z2).astype(np.int32)
        slot_node = meta["slot_nodes"][c]
        invd = np.ones((128, PR, F), np.float32)
        real = slot_node >= 0
        rr = np.arange(SLOTS)
        invd[rr[real] % 128, rr[real] // 128, :] = inv[slot_node[real]][:, None]
        xfm = np.zeros((F, SLOTS), np.float32)
        xfm[:, rr[real]] = x[slot_node[real]].T
        per_core.append({"goff1": off1, "goff2": g2, "invd": invd, "xfm": xfm,
                         "htbl": htbl})
    return per_core


# ---------------------------------------------------------------- bass builder

def _build_bass(meta, n_cores=P):
    from concourse import bacc, tile, mybir
    from concourse.bass import IndirectOffsetOnAxis

    N, NP, PR, SLOTS, C = (meta[k] for k in ("N", "NP", "PR", "SLOTS", "C"))
    T1 = N + 128
    T2 = P * SLOTS
    f32 = mybir.dt.float32
    AF = mybir.ActivationFunctionType
    OP = mybir.AluOpType
    AX = mybir.AxisListType

    nc = bacc.Bacc("TRN2", target_bir_lowering=False, debug=False,
                   num_devices=n_cores)
    htbl = nc.dram_tensor("htbl", [T1, F], f32, kind="ExternalInput")
    goff1 = nc.dram_tensor("goff1", [128, C], mybir.dt.int32, kind="ExternalInput")
    goff2 = nc.dram_tensor("goff2", [128, C], mybir.dt.int32, kind="ExternalInput")
    invd_d = nc.dram_tensor("invd", [128, PR, F], f32, kind="ExternalInput")
    xfm_d = nc.dram_tensor("xfm", [F, SLOTS], f32, kind="ExternalInput")
    ident_d = nc.dram_tensor("ident", [128, 128], f32, kind="ExternalInput")
    wts = {}
    for i in (1, 2, 3):
        wts[f"Wl{i}"] = nc.dram_tensor(f"Wl{i}", [F, F], f32, kind="ExternalInput")
        wts[f"Wr{i}"] = nc.dram_tensor(f"Wr{i}", [F, F], f32, kind="ExternalInput")
        wts[f"bl{i}"] = nc.dram_tensor(f"bl{i}", [F, 1], f32, kind="ExternalInput")
    wts["Wc"] = nc.dram_tensor("Wc", [F, NCLS], f32, kind="ExternalInput")
    wts["bc"] = nc.dram_tensor("bc", [128, NCLS], f32, kind="ExternalInput")
    out_d = nc.dram_tensor("out", [SLOTS, NCLS], f32, kind="ExternalOutput")

    n_mm = (SLOTS + MMW - 1) // MMW

    with tile.TileContext(nc) as tc:
        from contextlib import ExitStack
        with ExitStack() as es:
            dram = es.enter_context(tc.tile_pool(name="dram", bufs=1, space="DRAM"))
            const = es.enter_context(tc.tile_pool(name="const", bufs=1))
            gbuf = es.enter_context(tc.tile_pool(name="gbuf", bufs=2))
            mpool = es.enter_context(tc.tile_pool(name="mpool", bufs=3))
            psT = es.enter_context(tc.tile_pool(name="psT", bufs=2, space="PSUM"))
            psM = es.enter_context(tc.tile_pool(name="psM", bufs=2, space="PSUM"))
            psN = es.enter_context(tc.tile_pool(name="psN", bufs=2, space="PSUM"))

            agin_t = dram.tile([SLOTS, F], f32, tag="agin", name="agin")
            agout_t = dram.tile([T2, F], f32, tag="agout", name="agout")
            hfm_a = dram.tile([F, SLOTS], f32, tag="hfm_a", name="hfm_a")
            hfm_b = dram.tile([F, SLOTS], f32, tag="hfm_b", name="hfm_b")
            hfm_dram = [hfm_a, hfm_b]

            goff_a = const.tile([128, C], mybir.dt.int32, tag="goff_a",
                                name="goff_a")
            goff_b = const.tile([128, C], mybir.dt.int32, tag="goff_b",
                                name="goff_b")
            goff_t = [goff_a, goff_b]
            nc.sync.dma_start(goff_a[:], goff1[:])
            nc.sync.dma_start(goff_b[:], goff2[:])
            invd_t = const.tile([128, PR, F], f32, tag="invd", name="invd_t")
            nc.sync.dma_start(invd_t[:], invd_d[:])
            ident_t = const.tile([128, 128], f32, tag="ident", name="ident_t")
            nc.sync.dma_start(ident_t[:], ident_d[:])
            w_t = {}
            for k, dten in wts.items():
                wtile = const.tile(list(dten.shape), f32, tag=k, name=f"w_{k}")
                w_t[k] = wtile
                nc.sync.dma_start(wtile[:], dten[:])

            mean_t = const.tile([128, PR, F], f32, tag="mean", name="mean_t")
            ngrid_t = const.tile([128, PR, F], f32, tag="ngrid", name="ngrid_t")
            ogrid_t = const.tile([128, PR, NCLS], f32, tag="ogrid",
                                 name="ogrid_t")
            ogrid2_t = const.tile([128, PR, NCLS], f32, tag="ogrid2",
                                  name="ogrid2_t")

            for L in range(3):
                table = htbl if L == 0 else agout_t
                go = goff_t[0] if L == 0 else goff_t[1]
                hin = xfm_d if L == 0 else hfm_dram[(L + 1) % 2]
                hout = hfm_dram[L % 2]
                Wl, Wr, bl = w_t[f"Wl{L+1}"], w_t[f"Wr{L+1}"], w_t[f"bl{L+1}"]

                for ch in meta["chunks"]:
                    W = ch["width"]
                    buf = gbuf.tile([128, CW, F], f32, tag="chunk", name="buf")
                    for j in range(W):
                        nc.gpsimd.indirect_dma_start(
                            buf[:, j, :], None, table[:],
                            IndirectOffsetOnAxis(
                                ap=go[:, ch["c0"] + j:ch["c0"] + j + 1], axis=0))
                    for r in ch["rects"]:
                        m, w, i0 = r["m"], r["w"], r["i0"]
                        srcap = buf[:, r["coff"]:r["coff"] + m * w, :]
                        if w == 1:
                            nc.vector.tensor_copy(mean_t[:, i0:i0 + m, :], srcap)
                        else:
                            v = srcap.rearrange("p (m w) f -> p m f w", m=m, w=w)
                            nc.vector.tensor_reduce(
                                mean_t[:, i0:i0 + m, :], v, AX.X, OP.add)
                nc.vector.tensor_mul(mean_t[:], mean_t[:], invd_t[:])

                for mm in range(n_mm):
                    s0 = mm * MMW
                    wd = min(MMW, SLOTS - s0)
                    npr = wd // 128
                    mfm = mpool.tile([F, MMW], f32, tag="mfm", name="mfm")
                    for k in range(npr):
                        i = s0 // 128 + k
                        ps = psT.tile([F, 128], f32, tag="psT", name="psTt")
                        nc.tensor.transpose(ps[:], mean_t[:, i, :], ident_t[:])
                        nc.scalar.activation(mfm[:, k * 128:(k + 1) * 128],
                                             ps[:], AF.Copy)
                    hin_sb = mpool.tile([F, MMW], f32, tag="hin", name="hin_sb")
                    nc.sync.dma_start(hin_sb[:, :wd], hin[:, s0:s0 + wd])
                    ps = psM.tile([F, MMW], f32, tag="psM", name="psMt")
                    nc.tensor.matmul(ps[:, :wd], Wl[:], mfm[:, :wd],
                                     start=True, stop=False)
                    nc.tensor.matmul(ps[:, :wd], Wr[:], hin_sb[:, :wd],
                                     start=False, stop=True)
                    hout_sb = mpool.tile([F, MMW], f32, tag="hout",
                                         name="hout_sb")
                    nc.scalar.activation(hout_sb[:, :wd], ps[:, :wd],
                                         AF.Relu, bias=bl[:])
                    if L == 2:
                        for k in range(npr):
                            i = s0 // 128 + k
                            psc = psN.tile([128, NCLS], f32, tag="psN",
                                           name="psct")
                            nc.tensor.matmul(
                                psc[:], hout_sb[:, k * 128:(k + 1) * 128],
                                w_t["Wc"][:], start=True, stop=True)
                            nc.vector.tensor_add(ogrid_t[:, i, :], psc[:],
                                                 w_t["bc"][:])
                    else:
                        if s0 + wd == SLOTS and SLOTS > NP:
                            zoff = NP - s0
                            nc.vector.memset(hout_sb[:, zoff:wd], 0.0)
                        nc.sync.dma_start(hout[:, s0:s0 + wd], hout_sb[:, :wd])
                        for k in range(npr):
                            i = s0 // 128 + k
                            psn = psN.tile([128, F], f32, tag="psN", name="psnt")
                            nc.tensor.transpose(
                                psn[:], hout_sb[:, k * 128:(k + 1) * 128],
                                ident_t[:F, :F])
                            nc.scalar.activation(ngrid_t[:, i, :], psn[:],
                                                 AF.Copy)

                if L < 2:
                    nc.sync.dma_start(
                        agin_t[:].rearrange("(i p) f -> p i f", p=128),
                        ngrid_t[:])
                    nc.gpsimd.collective_compute(
                        "AllGather", OP.bypass,
                        ins=[agin_t.opt()], outs=[agout_t.opt()],
                        replica_groups=[list(range(n_cores))])

            mx = const.tile([128, PR, 1], f32, tag="mx", name="mx")
            nc.vector.tensor_reduce(mx[:], ogrid_t[:], AX.X, OP.max)
            nc.vector.tensor_sub(ogrid2_t[:], ogrid_t[:],
                                 mx[:].broadcast_to([128, PR, NCLS]))
            eg = const.tile([128, PR, NCLS], f32, tag="eg", name="eg")
            nc.scalar.activation(eg[:], ogrid2_t[:], AF.Exp)
            sm = const.tile([128, PR, 1], f32, tag="sm", name="sm")
            nc.vector.tensor_reduce(sm[:], eg[:], AX.X, OP.add)
            lsm = const.tile([128, PR, 1], f32, tag="lsm", name="lsm")
            nc.scalar.activation(lsm[:], sm[:], AF.Ln)
            nc.vector.tensor_sub(ogrid_t[:], ogrid2_t[:],
                                 lsm[:].broadcast_to([128, PR, NCLS]))
            nc.sync.dma_start(out_d[:].rearrange("(i p) c -> p i c", p=128),
                              ogrid_t[:])
    nc.compile()
    return nc


def _install_ntff_hook():
    mod = types.ModuleType("antenv.axon_hooks")
    def s(h):
        mod._hook = h
    def g():
        return getattr(mod, "_hook", None)
    mod.set_axon_ntff_profile_hook = s
    mod.get_axon_ntff_profile_hook = g
    sys.modules["antenv.axon_hooks"] = mod
    import antenv
    antenv.axon_hooks = mod
    from trn_agent_boot.trn_boot import _ntff_profile_via_ctypes
    s(_ntff_profile_via_ctypes("/opt/axon/libaxon_pjrt.so"))


def kernel(**inputs):
    global LAST_EXEC_NS
    from concourse import bass_utils
    from concourse.bass_interp import get_hw_module

    x = np.asarray(inputs["x"], np.float32)
    edge_index = np.asarray(inputs["edge_index"], np.int64)
    N = x.shape[0]

    meta = _build_meta(edge_index, N)
    per_core = _build_core_inputs(meta, x)
    nc = _build_bass(meta, n_cores=P)
    nc.m = get_hw_module(nc.m)

    ident = np.eye(128, dtype=np.float32)
    ins = []
    for c in range(P):
        pc = per_core[c]
        m = {"htbl": pc["htbl"], "goff1": pc["goff1"], "goff2": pc["goff2"],
             "invd": pc["invd"], "xfm": pc["xfm"], "ident": ident}
        for i in (1, 2, 3):
            m[f"Wl{i}"] = np.asarray(inputs[f"Wl{i}"], np.float32)
            m[f"Wr{i}"] = np.asarray(inputs[f"Wr{i}"], np.float32)
            m[f"bl{i}"] = np.asarray(inputs[f"bl{i}"],
                                     np.float32).reshape(F, 1)
        m["Wc"] = np.asarray(inputs["Wc"], np.float32)
        m["bc"] = np.tile(np.asarray(inputs["bc"], np.float32).reshape(1, NCLS),
                          (128, 1))
        ins.append(m)

    trace = os.environ.get("KERNEL_TRACE", "0") == "1"
    if trace:
        try:
            _install_ntff_hook()
        except Exception:
            trace = False
    res = bass_utils.run_bass_kernel_spmd(
        nc, ins, core_ids=list(range(P)), trace=trace)
    LAST_EXEC_NS = res.exec_time_ns

    full = np.zeros((N, NCLS), np.float32)
    for c in range(P):
        sn = meta["slot_nodes"][c]
        real = sn >= 0
        full[sn[real]] = res.results[c]["out"][real]
    return full


# revision 3
# speedup vs baseline: 1.0623x; 1.0623x over previous
"""GraphSAGE (3x SAGEConv mean-agg + linear classifier + log_softmax) on 8
Trainium2 NeuronCores via Bass.

Self-contained: host-side packing + SPMD bass program + gather/unshard.

Sharding: nodes are dst-sharded 8 ways (core c owns nodes [c*NP, (c+1)*NP)).
Per layer, each core:
  - gathers its in-edges' source rows from a replicated DRAM feature table
    with [128,1]-offset indirect DMAs (one 128-token grid column per call),
  - segment-reduces degree-sorted "prow" rectangles on the Vector engine
    (one strided tensor_reduce per rectangle), scales by 1/deg,
  - PE-transposes the mean grid to feature-major, matmuls Wl/Wr with PSUM
    accumulation, applies bias+ReLU on the Scalar engine,
  - PE-transposes back to node-major and AllGathers the shard into the next
    layer's table (compute/slot order).
The tiny 64-wide linears are replicated on every core.
"""
import os
import sys
import types

import numpy as np

sys.path.insert(0, "/opt/trn_rl_repo")

P = 8
F = 64
NCLS = 10
CW = 64          # gather-chunk width (grid columns per SBUF chunk buffer)
MMW = 512        # matmul moving-chunk width

LAST_EXEC_NS = None


# ---------------------------------------------------------------- host packing

def _build_meta(edge_index, n_nodes):
    N = n_nodes
    NP = N // P
    PR = (NP + 127) // 128
    SLOTS = PR * 128
    assert NP < SLOTS, "need at least one dummy slot (NP not divisible by 128)"
    src = np.asarray(edge_index[0], dtype=np.int64)
    dst = np.asarray(edge_index[1], dtype=np.int64)
    deg = np.bincount(dst, minlength=N).astype(np.int64)

    orders = []
    deg_sorted = []
    for c in range(P):
        dl = deg[c * NP:(c + 1) * NP]
        o = np.argsort(-dl, kind="stable")
        orders.append(o)
        deg_sorted.append(dl[o])
    w_prow = np.zeros(PR, dtype=np.int64)
    for i in range(PR):
        w_prow[i] = max(1, max(int(ds[i * 128]) for ds in deg_sorted))

    chunks = []
    col = 0
    cur = {"c0": 0, "width": 0, "rects": []}
    i = 0
    while i < PR:
        w = int(w_prow[i])
        if cur["width"] + w > CW and cur["width"] > 0:
            chunks.append(cur)
            cur = {"c0": col, "width": 0, "rects": []}
        m = 0
        while (i + m < PR and int(w_prow[i + m]) == w
               and cur["width"] + (m + 1) * w <= CW):
            m += 1
        if m == 0:
            assert w <= CW, f"prow width {w} exceeds chunk width {CW}"
            m = 1
        cur["rects"].append({"coff": cur["width"], "i0": i, "m": m, "w": w})
        cur["width"] += m * w
        col += m * w
        i += m
    if cur["width"] > 0:
        chunks.append(cur)
    C = col

    cc = 0
    for ch in chunks:
        ch["c0"] = cc
        cc += ch["width"]
    c0_prow = np.zeros(PR, dtype=np.int64)
    for ch in chunks:
        for r in ch["rects"]:
            for k in range(r["m"]):
                c0_prow[r["i0"] + k] = ch["c0"] + r["coff"] + k * r["w"]

    grids = []
    slot_nodes = []
    slot_of = np.full(N, -1, dtype=np.int64)
    for c in range(P):
        o = orders[c]
        slot_node = np.full(SLOTS, -1, dtype=np.int64)
        slot_node[:NP] = o + c * NP
        slot_nodes.append(slot_node)
        slot_of[o + c * NP] = np.arange(NP)

        grid = np.full((128, C), -1, dtype=np.int64)
        m = (dst >= c * NP) & (dst < (c + 1) * NP)
        es, ed = src[m], dst[m] - c * NP
        eo = np.argsort(ed, kind="stable")
        es, ed = es[eo], ed[eo]
        estart = np.zeros(NP + 1, dtype=np.int64)
        np.cumsum(np.bincount(ed, minlength=NP), out=estart[1:])
        r_e = slot_of[ed + c * NP]
        k_e = np.arange(es.shape[0]) - estart[ed]
        pp = r_e % 128
        cols = c0_prow[r_e // 128] + k_e
        grid[pp, cols] = es
        grids.append(grid)

    return {
        "N": N, "NP": NP, "PR": PR, "SLOTS": SLOTS, "C": C,
        "chunks": chunks, "grids": grids, "slot_nodes": slot_nodes,
        "slot_of": slot_of, "deg": deg,
    }


def _build_core_inputs(meta, x):
    N, NP, PR, SLOTS, C = (meta[k] for k in ("N", "NP", "PR", "SLOTS", "C"))
    Z1 = N
    Z2 = NP          # core 0's first dummy slot (zeroed on device)
    inv = 1.0 / np.maximum(meta["deg"], 1).astype(np.float32)

    htbl = np.zeros((N + 128, F), np.float32)
    htbl[:N] = x

    per_core = []
    for c in range(P):
        grid = meta["grids"][c]
        off1 = np.where(grid >= 0, grid, Z1).astype(np.int32)
        g2 = np.where(grid >= 0,
                      (grid // NP) * SLOTS + meta["slot_of"][np.maximum(grid, 0)],
                      Z2).astype(np.int32)
        slot_node = meta["slot_nodes"][c]
        invd = np.ones((128, PR, F), np.float32)
        real = slot_node >= 0
        rr = np.arange(SLOTS)
        invd[rr[real] % 128, rr[real] // 128, :] = inv[slot_node[real]][:, None]
        xfm = np.zeros((F, SLOTS), np.float32)
        xfm[:, rr[real]] = x[slot_node[real]].T
        per_core.append({"goff1": off1, "goff2": g2, "invd": invd, "xfm": xfm,
                         "htbl": htbl})
    return per_core


# ---------------------------------------------------------------- bass builder

def _build_bass(meta, n_cores=P):
    from concourse import bacc, tile, mybir
    from concourse.bass import IndirectOffsetOnAxis

    N, NP, PR, SLOTS, C = (meta[k] for k in ("N", "NP", "PR", "SLOTS", "C"))
    T1 = N + 128
    T2 = P * SLOTS
    f32 = mybir.dt.float32
    AF = mybir.ActivationFunctionType
    OP = mybir.AluOpType
    AX = mybir.AxisListType

    nc = bacc.Bacc("TRN2", target_bir_lowering=False, debug=False,
                   num_devices=n_cores)
    htbl = nc.dram_tensor("htbl", [T1, F], f32, kind="ExternalInput")
    goff1 = nc.dram_tensor("goff1", [128, C], mybir.dt.int32, kind="ExternalInput")
    goff2 = nc.dram_tensor("goff2", [128, C], mybir.dt.int32, kind="ExternalInput")
    invd_d = nc.dram_tensor("invd", [128, PR, F], f32, kind="ExternalInput")
    xfm_d = nc.dram_tensor("xfm", [F, SLOTS], f32, kind="ExternalInput")
    ident_d = nc.dram_tensor("ident", [128, 128], f32, kind="ExternalInput")
    wts = {}
    for i in (1, 2, 3):
        wts[f"Wl{i}"] = nc.dram_tensor(f"Wl{i}", [F, F], f32, kind="ExternalInput")
        wts[f"Wr{i}"] = nc.dram_tensor(f"Wr{i}", [F, F], f32, kind="ExternalInput")
        wts[f"bl{i}"] = nc.dram_tensor(f"bl{i}", [F, 1], f32, kind="ExternalInput")
    wts["Wc"] = nc.dram_tensor("Wc", [F, NCLS], f32, kind="ExternalInput")
    wts["bc"] = nc.dram_tensor("bc", [128, NCLS], f32, kind="ExternalInput")
    out_d = nc.dram_tensor("out", [SLOTS, NCLS], f32, kind="ExternalOutput")

    maxpr = max(ch["rects"][-1]["i0"] + ch["rects"][-1]["m"]
                - ch["rects"][0]["i0"] for ch in meta["chunks"])

    with tile.TileContext(nc) as tc:
        from contextlib import ExitStack
        with ExitStack() as es:
            dram = es.enter_context(tc.tile_pool(name="dram", bufs=1, space="DRAM"))
            const = es.enter_context(tc.tile_pool(name="const", bufs=1))
            gbuf = es.enter_context(tc.tile_pool(name="gbuf", bufs=2))
            mpool = es.enter_context(tc.tile_pool(name="mpool", bufs=3))
            psT = es.enter_context(tc.tile_pool(name="psT", bufs=2, space="PSUM"))
            psM = es.enter_context(tc.tile_pool(name="psM", bufs=2, space="PSUM"))
            psN = es.enter_context(tc.tile_pool(name="psN", bufs=2, space="PSUM"))

            agin_t = dram.tile([SLOTS, F], f32, tag="agin", name="agin")
            agout_t = dram.tile([T2, F], f32, tag="agout", name="agout")
            hfm_a = dram.tile([F, SLOTS], f32, tag="hfm_a", name="hfm_a")
            hfm_b = dram.tile([F, SLOTS], f32, tag="hfm_b", name="hfm_b")
            hfm_dram = [hfm_a, hfm_b]

            goff_a = const.tile([128, C], mybir.dt.int32, tag="goff_a",
                                name="goff_a")
            goff_b = const.tile([128, C], mybir.dt.int32, tag="goff_b",
                                name="goff_b")
            goff_t = [goff_a, goff_b]
            nc.sync.dma_start(goff_a[:], goff1[:])
            nc.sync.dma_start(goff_b[:], goff2[:])
            invd_t = const.tile([128, PR, F], f32, tag="invd", name="invd_t")
            nc.sync.dma_start(invd_t[:], invd_d[:])
            ident_t = const.tile([128, 128], f32, tag="ident", name="ident_t")
            nc.sync.dma_start(ident_t[:], ident_d[:])
            w_t = {}
            for k, dten in wts.items():
                wtile = const.tile(list(dten.shape), f32, tag=k, name=f"w_{k}")
                w_t[k] = wtile
                nc.sync.dma_start(wtile[:], dten[:])

            mean_t = const.tile([128, PR, F], f32, tag="mean", name="mean_t")
            ngrid_t = const.tile([128, PR, F], f32, tag="ngrid", name="ngrid_t")
            ogrid_t = const.tile([128, PR, NCLS], f32, tag="ogrid",
                                 name="ogrid_t")
            ogrid2_t = const.tile([128, PR, NCLS], f32, tag="ogrid2",
                                  name="ogrid2_t")

            for L in range(3):
                table = htbl if L == 0 else agout_t
                go = goff_t[0] if L == 0 else goff_t[1]
                hin = xfm_d if L == 0 else hfm_dram[(L + 1) % 2]
                hout = hfm_dram[L % 2]
                Wl, Wr, bl = w_t[f"Wl{L+1}"], w_t[f"Wr{L+1}"], w_t[f"bl{L+1}"]

                for ch in meta["chunks"]:
                    W = ch["width"]
                    buf = gbuf.tile([128, CW, F], f32, tag="chunk", name="buf")
                    for j in range(W):
                        nc.gpsimd.indirect_dma_start(
                            buf[:, j, :], None, table[:],
                            IndirectOffsetOnAxis(
                                ap=go[:, ch["c0"] + j:ch["c0"] + j + 1], axis=0))
                    for r in ch["rects"]:
                        m, w, i0 = r["m"], r["w"], r["i0"]
                        srcap = buf[:, r["coff"]:r["coff"] + m * w, :]
                        if w == 1:
                            nc.vector.tensor_copy(mean_t[:, i0:i0 + m, :], srcap)
                        else:
                            v = srcap.rearrange("p (m w) f -> p m f w", m=m, w=w)
                            nc.vector.tensor_reduce(
                                mean_t[:, i0:i0 + m, :], v, AX.X, OP.add)
                    # this chunk's prows are final: scale, transpose, matmul now
                    i0c = ch["rects"][0]["i0"]
                    i1c = ch["rects"][-1]["i0"] + ch["rects"][-1]["m"]
                    npr = i1c - i0c
                    s0 = i0c * 128
                    wd = npr * 128
                    nc.vector.tensor_mul(mean_t[:, i0c:i1c, :],
                                         mean_t[:, i0c:i1c, :],
                                         invd_t[:, i0c:i1c, :])
                    mfm = mpool.tile([F, maxpr * 128], f32, tag="mfm", name="mfm")
                    for k in range(npr):
                        i = i0c + k
                        ps = psT.tile([F, 128], f32, tag="psT", name="psTt")
                        nc.tensor.transpose(ps[:], mean_t[:, i, :], ident_t[:])
                        nc.scalar.activation(mfm[:, k * 128:(k + 1) * 128],
                                             ps[:], AF.Copy)
                    hin_sb = mpool.tile([F, maxpr * 128], f32, tag="hin",
                                        name="hin_sb")
                    nc.sync.dma_start(hin_sb[:, :wd], hin[:, s0:s0 + wd])
                    for q0 in range(0, wd, MMW):
                        qw = min(MMW, wd - q0)
                        ps = psM.tile([F, MMW], f32, tag="psM", name="psMt")
                        nc.tensor.matmul(ps[:, :qw], Wl[:],
                                         mfm[:, q0:q0 + qw],
                                         start=True, stop=False)
                        nc.tensor.matmul(ps[:, :qw], Wr[:],
                                         hin_sb[:, q0:q0 + qw],
                                         start=False, stop=True)
                        nc.scalar.activation(mfm[:, q0:q0 + qw], ps[:, :qw],
                                             AF.Relu, bias=bl[:])
                    hout_sb = mfm   # relu result written back into mfm tile
                    if L == 2:
                        for k in range(npr):
                            i = i0c + k
                            psc = psN.tile([128, NCLS], f32, tag="psN",
                                           name="psct")
                            nc.tensor.matmul(
                                psc[:], hout_sb[:, k * 128:(k + 1) * 128],
                                w_t["Wc"][:], start=True, stop=True)
                            nc.vector.tensor_add(ogrid_t[:, i, :], psc[:],
                                                 w_t["bc"][:])
                    else:
                        if s0 + wd > NP:
                            zoff = max(0, NP - s0)
                            nc.vector.memset(hout_sb[:, zoff:wd], 0.0)
                        nc.sync.dma_start(hout[:, s0:s0 + wd], hout_sb[:, :wd])
                        for k in range(npr):
                            i = i0c + k
                            psn = psN.tile([128, F], f32, tag="psN", name="psnt")
                            nc.tensor.transpose(
                                psn[:], hout_sb[:, k * 128:(k + 1) * 128],
                                ident_t[:F, :F])
                            nc.scalar.activation(ngrid_t[:, i, :], psn[:],
                                                 AF.Copy)

                if L < 2:
                    nc.sync.dma_start(
                        agin_t[:].rearrange("(i p) f -> p i f", p=128),
                        ngrid_t[:])
                    nc.gpsimd.collective_compute(
                        "AllGather", OP.bypass,
                        ins=[agin_t.opt()], outs=[agout_t.opt()],
                        replica_groups=[list(range(n_cores))])

            mx = const.tile([128, PR, 1], f32, tag="mx", name="mx")
            nc.vector.tensor_reduce(mx[:], ogrid_t[:], AX.X, OP.max)
            nc.vector.tensor_sub(ogrid2_t[:], ogrid_t[:],
                                 mx[:].broadcast_to([128, PR, NCLS]))
            eg = const.tile([128, PR, NCLS], f32, tag="eg", name="eg")
            nc.scalar.activation(eg[:], ogrid2_t[:], AF.Exp)
            sm = const.tile([128, PR, 1], f32, tag="sm", name="sm")
            nc.vector.tensor_reduce(sm[:], eg[:], AX.X, OP.add)
            lsm = const.tile([128, PR, 1], f32, tag="lsm", name="lsm")
            nc.scalar.activation(lsm[:], sm[:], AF.Ln)
            nc.vector.tensor_sub(ogrid_t[:], ogrid2_t[:],
                                 lsm[:].broadcast_to([128, PR, NCLS]))
            nc.sync.dma_start(out_d[:].rearrange("(i p) c -> p i c", p=128),
                              ogrid_t[:])
    nc.compile()
    return nc


def _install_ntff_hook():
    mod = types.ModuleType("antenv.axon_hooks")
    def s(h):
        mod._hook = h
    def g():
        return getattr(mod, "_hook", None)
    mod.set_axon_ntff_profile_hook = s
    mod.get_axon_ntff_profile_hook = g
    sys.modules["antenv.axon_hooks"] = mod
    import antenv
    antenv.axon_hooks = mod
    from trn_agent_boot.trn_boot import _ntff_profile_via_ctypes
    s(_ntff_profile_via_ctypes("/opt/axon/libaxon_pjrt.so"))


def kernel(**inputs):
    global LAST_EXEC_NS
    from concourse import bass_utils
    from concourse.bass_interp import get_hw_module

    x = np.asarray(inputs["x"], np.float32)
    edge_index = np.asarray(inputs["edge_index"], np.int64)
    N = x.shape[0]

    meta = _build_meta(edge_index, N)
    per_core = _build_core_inputs(meta, x)
    nc = _build_bass(meta, n_cores=P)
    nc.m = get_hw_module(nc.m)

    ident = np.eye(128, dtype=np.float32)
    ins = []
    for c in range(P):
        pc = per_core[c]
        m = {"htbl": pc["htbl"], "goff1": pc["goff1"], "goff2": pc["goff2"],
             "invd": pc["invd"], "xfm": pc["xfm"], "ident": ident}
        for i in (1, 2, 3):
            m[f"Wl{i}"] = np.asarray(inputs[f"Wl{i}"], np.float32)
            m[f"Wr{i}"] = np.asarray(inputs[f"Wr{i}"], np.float32)
            m[f"bl{i}"] = np.asarray(inputs[f"bl{i}"],
                                     np.float32).reshape(F, 1)
        m["Wc"] = np.asarray(inputs["Wc"], np.float32)
        m["bc"] = np.tile(np.asarray(inputs["bc"], np.float32).reshape(1, NCLS),
                          (128, 1))
        ins.append(m)

    trace = os.environ.get("KERNEL_TRACE", "0") == "1"
    if trace:
        try:
            _install_ntff_hook()
        except Exception:
            trace = False
    res = bass_utils.run_bass_kernel_spmd(
        nc, ins, core_ids=list(range(P)), trace=trace)
    LAST_EXEC_NS = res.exec_time_ns

    full = np.zeros((N, NCLS), np.float32)
    for c in range(P):
        sn = meta["slot_nodes"][c]
        real = sn >= 0
        full[sn[real]] = res.results[c]["out"][real]
    return full


# revision 5
# speedup vs baseline: 1.0964x; 1.0321x over previous
"""GraphSAGE (3x SAGEConv mean-agg + linear classifier + log_softmax) on 8
Trainium2 NeuronCores via Bass.

Self-contained: host-side packing + SPMD bass program + gather/unshard.

Sharding: nodes are dst-sharded 8 ways (core c owns nodes [c*NP, (c+1)*NP)).
Per layer, each core:
  - gathers its in-edges' source rows from a replicated DRAM feature table
    with [128,1]-offset indirect DMAs (one 128-token grid column per call),
  - segment-reduces degree-sorted "prow" rectangles on the Vector engine
    (one strided tensor_reduce per rectangle), scales by 1/deg,
  - PE-transposes the mean grid to feature-major, matmuls Wl/Wr with PSUM
    accumulation, applies bias+ReLU on the Scalar engine,
  - PE-transposes back to node-major and AllGathers the shard into the next
    layer's table (compute/slot order).
The tiny 64-wide linears are replicated on every core.
"""
import os
import sys
import types

import numpy as np

sys.path.insert(0, "/opt/trn_rl_repo")

P = 8
F = 64
NCLS = 10
CW = 64          # gather-chunk width (grid columns per SBUF chunk buffer)
MMW = 512        # matmul moving-chunk width

LAST_EXEC_NS = None


# ---------------------------------------------------------------- host packing

def _build_meta(edge_index, n_nodes):
    N = n_nodes
    NP = N // P
    PR = (NP + 127) // 128
    SLOTS = PR * 128
    assert NP < SLOTS, "need at least one dummy slot (NP not divisible by 128)"
    src = np.asarray(edge_index[0], dtype=np.int64)
    dst = np.asarray(edge_index[1], dtype=np.int64)
    deg = np.bincount(dst, minlength=N).astype(np.int64)

    orders = []
    deg_sorted = []
    for c in range(P):
        dl = deg[c * NP:(c + 1) * NP]
        o = np.argsort(-dl, kind="stable")
        orders.append(o)
        deg_sorted.append(dl[o])
    w_prow = np.zeros(PR, dtype=np.int64)
    for i in range(PR):
        w_prow[i] = max(1, max(int(ds[i * 128]) for ds in deg_sorted))

    chunks = []
    col = 0
    cur = {"c0": 0, "width": 0, "rects": []}
    i = 0
    while i < PR:
        w = int(w_prow[i])
        if cur["width"] + w > CW and cur["width"] > 0:
            chunks.append(cur)
            cur = {"c0": col, "width": 0, "rects": []}
        m = 0
        while (i + m < PR and int(w_prow[i + m]) == w
               and cur["width"] + (m + 1) * w <= CW):
            m += 1
        if m == 0:
            assert w <= CW, f"prow width {w} exceeds chunk width {CW}"
            m = 1
        cur["rects"].append({"coff": cur["width"], "i0": i, "m": m, "w": w})
        cur["width"] += m * w
        col += m * w
        i += m
    if cur["width"] > 0:
        chunks.append(cur)
    C = col

    cc = 0
    for ch in chunks:
        ch["c0"] = cc
        cc += ch["width"]
    c0_prow = np.zeros(PR, dtype=np.int64)
    for ch in chunks:
        for r in ch["rects"]:
            for k in range(r["m"]):
                c0_prow[r["i0"] + k] = ch["c0"] + r["coff"] + k * r["w"]

    grids = []
    slot_nodes = []
    slot_of = np.full(N, -1, dtype=np.int64)
    for c in range(P):
        o = orders[c]
        slot_node = np.full(SLOTS, -1, dtype=np.int64)
        slot_node[:NP] = o + c * NP
        slot_nodes.append(slot_node)
        slot_of[o + c * NP] = np.arange(NP)

        grid = np.full((128, C), -1, dtype=np.int64)
        m = (dst >= c * NP) & (dst < (c + 1) * NP)
        es, ed = src[m], dst[m] - c * NP
        eo = np.argsort(ed, kind="stable")
        es, ed = es[eo], ed[eo]
        estart = np.zeros(NP + 1, dtype=np.int64)
        np.cumsum(np.bincount(ed, minlength=NP), out=estart[1:])
        r_e = slot_of[ed + c * NP]
        k_e = np.arange(es.shape[0]) - estart[ed]
        pp = r_e % 128
        cols = c0_prow[r_e // 128] + k_e
        grid[pp, cols] = es
        grids.append(grid)

    return {
        "N": N, "NP": NP, "PR": PR, "SLOTS": SLOTS, "C": C,
        "chunks": chunks, "grids": grids, "slot_nodes": slot_nodes,
        "slot_of": slot_of, "deg": deg,
    }


def _build_core_inputs(meta, x):
    N, NP, PR, SLOTS, C = (meta[k] for k in ("N", "NP", "PR", "SLOTS", "C"))
    Z1 = N
    Z2 = NP          # core 0's first dummy slot (zeroed on device)
    inv = 1.0 / np.maximum(meta["deg"], 1).astype(np.float32)

    htbl = np.zeros((N + 128, F), np.float32)
    htbl[:N] = x

    per_core = []
    for c in range(P):
        grid = meta["grids"][c]
        off1 = np.where(grid >= 0, grid, Z1).astype(np.int32)
        H = (PR // 2) * 128
        owner = grid // NP
        slot = meta["slot_of"][np.maximum(grid, 0)]
        gid = np.where(slot < H, owner * H + slot,
                       P * H + owner * (SLOTS - H) + (slot - H))
        z2slot = NP
        z2 = (z2slot < H) and Z2 or (P * H + 0 * (SLOTS - H) + (z2slot - H))
        z2 = int(z2)
        g2 = np.where(grid >= 0, gid, z2).astype(np.int32)
        slot_node = meta["slot_nodes"][c]
        invd = np.ones((128, PR, F), np.float32)
        real = slot_node >= 0
        rr = np.arange(SLOTS)
        invd[rr[real] % 128, rr[real] // 128, :] = inv[slot_node[real]][:, None]
        xfm = np.zeros((F, SLOTS), np.float32)
        xfm[:, rr[real]] = x[slot_node[real]].T
        per_core.append({"goff1": off1, "goff2": g2, "invd": invd, "xfm": xfm,
                         "htbl": htbl})
    return per_core


# ---------------------------------------------------------------- bass builder

def _build_bass(meta, n_cores=P):
    from concourse import bacc, tile, mybir
    from concourse.bass import IndirectOffsetOnAxis

    N, NP, PR, SLOTS, C = (meta[k] for k in ("N", "NP", "PR", "SLOTS", "C"))
    T1 = N + 128
    T2 = P * SLOTS
    f32 = mybir.dt.float32
    AF = mybir.ActivationFunctionType
    OP = mybir.AluOpType
    AX = mybir.AxisListType

    nc = bacc.Bacc("TRN2", target_bir_lowering=False, debug=False,
                   num_devices=n_cores)
    htbl = nc.dram_tensor("htbl", [T1, F], f32, kind="ExternalInput")
    goff1 = nc.dram_tensor("goff1", [128, C], mybir.dt.int32, kind="ExternalInput")
    goff2 = nc.dram_tensor("goff2", [128, C], mybir.dt.int32, kind="ExternalInput")
    invd_d = nc.dram_tensor("invd", [128, PR, F], f32, kind="ExternalInput")
    xfm_d = nc.dram_tensor("xfm", [F, SLOTS], f32, kind="ExternalInput")
    ident_d = nc.dram_tensor("ident", [128, 128], f32, kind="ExternalInput")
    wts = {}
    for i in (1, 2, 3):
        wts[f"Wl{i}"] = nc.dram_tensor(f"Wl{i}", [F, F], f32, kind="ExternalInput")
        wts[f"Wr{i}"] = nc.dram_tensor(f"Wr{i}", [F, F], f32, kind="ExternalInput")
        wts[f"bl{i}"] = nc.dram_tensor(f"bl{i}", [F, 1], f32, kind="ExternalInput")
    wts["Wc"] = nc.dram_tensor("Wc", [F, NCLS], f32, kind="ExternalInput")
    wts["bc"] = nc.dram_tensor("bc", [128, NCLS], f32, kind="ExternalInput")
    out_d = nc.dram_tensor("out", [SLOTS, NCLS], f32, kind="ExternalOutput")

    maxpr = max(ch["rects"][-1]["i0"] + ch["rects"][-1]["m"]
                - ch["rects"][0]["i0"] for ch in meta["chunks"])
    HPR = PR // 2
    H = HPR * 128

    with tile.TileContext(nc) as tc:
        from contextlib import ExitStack
        with ExitStack() as es:
            dram = es.enter_context(tc.tile_pool(name="dram", bufs=1, space="DRAM"))
            const = es.enter_context(tc.tile_pool(name="const", bufs=1))
            gbuf = es.enter_context(tc.tile_pool(name="gbuf", bufs=2))
            mpool = es.enter_context(tc.tile_pool(name="mpool", bufs=3))
            psT = es.enter_context(tc.tile_pool(name="psT", bufs=2, space="PSUM"))
            psM = es.enter_context(tc.tile_pool(name="psM", bufs=2, space="PSUM"))
            psN = es.enter_context(tc.tile_pool(name="psN", bufs=2, space="PSUM"))

            agin_t = dram.tile([SLOTS, F], f32, tag="agin", name="agin")
            agout_a = dram.tile([T2, F], f32, tag="agout_a", name="agout_a")
            agout_b = dram.tile([T2, F], f32, tag="agout_b", name="agout_b")
            hfm_a = dram.tile([F, SLOTS], f32, tag="hfm_a", name="hfm_a")
            hfm_b = dram.tile([F, SLOTS], f32, tag="hfm_b", name="hfm_b")
            hfm_dram = [hfm_a, hfm_b]

            goff_a = const.tile([128, C], mybir.dt.int32, tag="goff_a",
                                name="goff_a")
            goff_b = const.tile([128, C], mybir.dt.int32, tag="goff_b",
                                name="goff_b")
            goff_t = [goff_a, goff_b]
            nc.sync.dma_start(goff_a[:], goff1[:])
            nc.sync.dma_start(goff_b[:], goff2[:])
            invd_t = const.tile([128, PR, F], f32, tag="invd", name="invd_t")
            nc.sync.dma_start(invd_t[:], invd_d[:])
            ident_t = const.tile([128, 128], f32, tag="ident", name="ident_t")
            nc.sync.dma_start(ident_t[:], ident_d[:])
            w_t = {}
            for k, dten in wts.items():
                wtile = const.tile(list(dten.shape), f32, tag=k, name=f"w_{k}")
                w_t[k] = wtile
                nc.sync.dma_start(wtile[:], dten[:])

            mean_t = const.tile([128, PR, F], f32, tag="mean", name="mean_t")
            ngrid_t = const.tile([128, PR, F], f32, tag="ngrid", name="ngrid_t")
            ogrid_t = const.tile([128, PR, NCLS], f32, tag="ogrid",
                                 name="ogrid_t")
            ogrid2_t = const.tile([128, PR, NCLS], f32, tag="ogrid2",
                                  name="ogrid2_t")

            for L in range(3):
                table = htbl if L == 0 else (agout_a if L == 1 else agout_b)
                agout_t = agout_a if L == 0 else agout_b
                go = goff_t[0] if L == 0 else goff_t[1]
                hin = xfm_d if L == 0 else hfm_dram[(L + 1) % 2]
                hout = hfm_dram[L % 2]
                Wl, Wr, bl = w_t[f"Wl{L+1}"], w_t[f"Wr{L+1}"], w_t[f"bl{L+1}"]

                half1_sent = False
                for ch in meta["chunks"]:
                    W = ch["width"]
                    buf = gbuf.tile([128, CW, F], f32, tag="chunk", name="buf")
                    for j in range(W):
                        nc.gpsimd.indirect_dma_start(
                            buf[:, j, :], None, table[:],
                            IndirectOffsetOnAxis(
                                ap=go[:, ch["c0"] + j:ch["c0"] + j + 1], axis=0))
                    for r in ch["rects"]:
                        m, w, i0 = r["m"], r["w"], r["i0"]
                        srcap = buf[:, r["coff"]:r["coff"] + m * w, :]
                        if w == 1:
                            nc.vector.tensor_copy(mean_t[:, i0:i0 + m, :], srcap)
                        else:
                            v = srcap.rearrange("p (m w) f -> p m f w", m=m, w=w)
                            nc.vector.tensor_reduce(
                                mean_t[:, i0:i0 + m, :], v, AX.X, OP.add)
                    # this chunk's prows are final: scale, transpose, matmul now
                    i0c = ch["rects"][0]["i0"]
                    i1c = ch["rects"][-1]["i0"] + ch["rects"][-1]["m"]
                    npr = i1c - i0c
                    s0 = i0c * 128
                    wd = npr * 128
                    nc.vector.tensor_mul(mean_t[:, i0c:i1c, :],
                                         mean_t[:, i0c:i1c, :],
                                         invd_t[:, i0c:i1c, :])
                    mfm = mpool.tile([F, maxpr * 128], f32, tag="mfm", name="mfm")
                    for k in range(npr):
                        i = i0c + k
                        ps = psT.tile([F, 128], f32, tag="psT", name="psTt")
                        nc.tensor.transpose(ps[:], mean_t[:, i, :], ident_t[:])
                        nc.scalar.activation(mfm[:, k * 128:(k + 1) * 128],
                                             ps[:], AF.Copy)
                    hin_sb = mpool.tile([F, maxpr * 128], f32, tag="hin",
                                        name="hin_sb")
                    nc.sync.dma_start(hin_sb[:, :wd], hin[:, s0:s0 + wd])
                    for q0 in range(0, wd, MMW):
                        qw = min(MMW, wd - q0)
                        ps = psM.tile([F, MMW], f32, tag="psM", name="psMt")
                        nc.tensor.matmul(ps[:, :qw], Wl[:],
                                         mfm[:, q0:q0 + qw],
                                         start=True, stop=False)
                        nc.tensor.matmul(ps[:, :qw], Wr[:],
                                         hin_sb[:, q0:q0 + qw],
                                         start=False, stop=True)
                        nc.scalar.activation(mfm[:, q0:q0 + qw], ps[:, :qw],
                                             AF.Relu, bias=bl[:])
                    hout_sb = mfm   # relu result written back into mfm tile
                    if L == 2:
                        for k in range(npr):
                            i = i0c + k
                            psc = psN.tile([128, NCLS], f32, tag="psN",
                                           name="psct")
                            nc.tensor.matmul(
                                psc[:], hout_sb[:, k * 128:(k + 1) * 128],
                                w_t["Wc"][:], start=True, stop=True)
                            nc.vector.tensor_add(ogrid_t[:, i, :], psc[:],
                                                 w_t["bc"][:])
                    else:
                        if s0 + wd > NP:
                            zoff = max(0, NP - s0)
                            nc.vector.memset(hout_sb[:, zoff:wd], 0.0)
                        nc.sync.dma_start(hout[:, s0:s0 + wd], hout_sb[:, :wd])
                        for k in range(npr):
                            i = i0c + k
                            psn = psN.tile([128, F], f32, tag="psN", name="psnt")
                            nc.tensor.transpose(
                                psn[:], hout_sb[:, k * 128:(k + 1) * 128],
                                ident_t[:F, :F])
                            nc.scalar.activation(ngrid_t[:, i, :], psn[:],
                                                 AF.Copy)
                        if not half1_sent and i1c >= HPR:
                            half1_sent = True
                            nc.sync.dma_start(
                                agin_t[0:H, :].rearrange(
                                    "(i p) f -> p i f", p=128),
                                ngrid_t[:, 0:HPR, :])
                            nc.gpsimd.collective_compute(
                                "AllGather", OP.bypass,
                                ins=[agin_t[0:H, :].opt()],
                                outs=[agout_t[0:n_cores * H, :].opt()],
                                replica_groups=[list(range(n_cores))])

                if L < 2:
                    nc.sync.dma_start(
                        agin_t[H:SLOTS, :].rearrange("(i p) f -> p i f", p=128),
                        ngrid_t[:, HPR:PR, :])
                    nc.gpsimd.collective_compute(
                        "AllGather", OP.bypass,
                        ins=[agin_t[H:SLOTS, :].opt()],
                        outs=[agout_t[n_cores * H:, :].opt()],
                        replica_groups=[list(range(n_cores))])

            mx = const.tile([128, PR, 1], f32, tag="mx", name="mx")
            nc.vector.tensor_reduce(mx[:], ogrid_t[:], AX.X, OP.max)
            nc.vector.tensor_sub(ogrid2_t[:], ogrid_t[:],
                                 mx[:].broadcast_to([128, PR, NCLS]))
            eg = const.tile([128, PR, NCLS], f32, tag="eg", name="eg")
            nc.scalar.activation(eg[:], ogrid2_t[:], AF.Exp)
            sm = const.tile([128, PR, 1], f32, tag="sm", name="sm")
            nc.vector.tensor_reduce(sm[:], eg[:], AX.X, OP.add)
            lsm = const.tile([128, PR, 1], f32, tag="lsm", name="lsm")
            nc.scalar.activation(lsm[:], sm[:], AF.Ln)
            nc.vector.tensor_sub(ogrid_t[:], ogrid2_t[:],
                                 lsm[:].broadcast_to([128, PR, NCLS]))
            nc.sync.dma_start(out_d[:].rearrange("(i p) c -> p i c", p=128),
                              ogrid_t[:])
    nc.compile()
    return nc


def _install_ntff_hook():
    mod = types.ModuleType("antenv.axon_hooks")
    def s(h):
        mod._hook = h
    def g():
        return getattr(mod, "_hook", None)
    mod.set_axon_ntff_profile_hook = s
    mod.get_axon_ntff_profile_hook = g
    sys.modules["antenv.axon_hooks"] = mod
    import antenv
    antenv.axon_hooks = mod
    from trn_agent_boot.trn_boot import _ntff_profile_via_ctypes
    s(_ntff_profile_via_ctypes("/opt/axon/libaxon_pjrt.so"))


def kernel(**inputs):
    global LAST_EXEC_NS
    from concourse import bass_utils
    from concourse.bass_interp import get_hw_module

    x = np.asarray(inputs["x"], np.float32)
    edge_index = np.asarray(inputs["edge_index"], np.int64)
    N = x.shape[0]

    meta = _build_meta(edge_index, N)
    per_core = _build_core_inputs(meta, x)
    nc = _build_bass(meta, n_cores=P)
    nc.m = get_hw_module(nc.m)

    ident = np.eye(128, dtype=np.float32)
    ins = []
    for c in range(P):
        pc = per_core[c]
        m = {"htbl": pc["htbl"], "goff1": pc["goff1"], "goff2": pc["goff2"],
             "invd": pc["invd"], "xfm": pc["xfm"], "ident": ident}
        for i in (1, 2, 3):
            m[f"Wl{i}"] = np.asarray(inputs[f"Wl{i}"], np.float32)
            m[f"Wr{i}"] = np.asarray(inputs[f"Wr{i}"], np.float32)
            m[f"bl{i}"] = np.asarray(inputs[f"bl{i}"],
                                     np.float32).reshape(F, 1)
        m["Wc"] = np.asarray(inputs["Wc"], np.float32)
        m["bc"] = np.tile(np.asarray(inputs["bc"], np.float32).reshape(1, NCLS),
                          (128, 1))
        ins.append(m)

    trace = os.environ.get("KERNEL_TRACE", "0") == "1"
    if trace:
        try:
            _install_ntff_hook()
        except Exception:
            trace = False
    res = bass_utils.run_bass_kernel_spmd(
        nc, ins, core_ids=list(range(P)), trace=trace)
    LAST_EXEC_NS = res.exec_time_ns

    full = np.zeros((N, NCLS), np.float32)
    for c in range(P):
        sn = meta["slot_nodes"][c]
        real = sn >= 0
        full[sn[real]] = res.results[c]["out"][real]
    return full
